# revision 24
# baseline (speedup 1.0000x reference)
# kernel.py — AtomTransformerBlock on 8 TRN2 NeuronCores (SPMD, no collectives).
#
# Sharding: N_atom rows across 8 cores (256 rows each); x + weights replicated
# (each core recomputes LN(x), K, V for all 2048 rows). pair_emb sharded by
# first axis. All index-derived masks are precomputed on the host (pure index
# preprocessing); all tensor math happens on device.
#
# Design — dense softmax (no score DRAM roundtrip):
#   pair-bias dma_gather (256B elements, 4 SWDGE queues in parallel; the
#   desc-gen ucode runs on Q7 core-pair `queue_num`, so 4 queues give ~4x
#   descriptor throughput) -> per-chunk channel reduce -> PMG[b, i] ->
#   xbar transpose -> local_scatter of pm values into dense-j space
#   (PMD[i, j], zeros elsewhere). Host feeds the dense log-multiplicity map
#   LM15D[i, j] = log(count)+EXPB at gathered j, 0 elsewhere. Scores stay
#   dense: S'[i,h,j] = q.k + lm15 (fused into the psum evac) + pm*wb[h], and
#   P = exp(S' - EXPB) kills non-gathered j. ACT accum_out yields the softmax
#   denominator during the exp; normalize is a per-partition tensor_scalar;
#   AV contracts dense P^T (DMA transpose) against dense V on PE with
#   col-tiled (M=32) matmuls, 4 heads per psum tile.
#   Emission interleaves gather chunks with score/evac pre-runs and pins the
#   DVE order with explicit deps (the scheduler's SWDGE cost model is ~6x
#   optimistic, so it would otherwise hoist all pair reduces to the front).
import math
import os
import sys

import numpy as np

sys.path.insert(0, "/opt/trn_rl_repo")

import ml_dtypes
from contextlib import ExitStack

import concourse.bass as bass
import concourse.mybir as mybir
import concourse.tile as tile
from concourse import bacc, library_config
from concourse.tile import add_dep_helper

N, C, H, DH, CP, B = 2048, 256, 8, 32, 32, 128
NCORES = 8
NSH = N // NCORES          # 256 rows per core
NBLK = NSH // 128          # 2 i-blocks per core
F32 = mybir.dt.float32
BF16 = mybir.dt.bfloat16
F16 = mybir.dt.float16
I16 = mybir.dt.int16
AX = mybir.AxisListType
ALU = mybir.AluOpType
AF = mybir.ActivationFunctionType

EXPB = 12.0                # dense-softmax exclusion offset


def build_nc():
    nc = bacc.Bacc(None, target_bir_lowering=False, debug=True, num_swdge_queues=4)

    x_d = nc.declare_dram_parameter("x", [N, C], F32, isOutput=False)
    xo_d = nc.declare_dram_parameter("xown", [NSH, C], F32, isOutput=False)
    pairs_d = nc.declare_dram_parameter("pairs", [NSH * 1024, 64], F32, isOutput=False)
    pgidx_d = nc.declare_dram_parameter("pgidx", [128, 8, 4, 64], I16, isOutput=False)
    pgpar_d = nc.declare_dram_parameter("pgpar", [128, 256], F32, isOutput=False)
    scidx_d = nc.declare_dram_parameter("scidx", [128, NBLK, 2, 128], I16, isOutput=False)
    lm15_d = nc.declare_dram_parameter("lm15", [128, NBLK, N], BF16, isOutput=False)
    ident_d = nc.declare_dram_parameter("ident", [128, 128], F32, isOutput=False)
    wqt_d = nc.declare_dram_parameter("wqt", [C, C], BF16, isOutput=False)
    wkt_d = nc.declare_dram_parameter("wkt", [C, C], BF16, isOutput=False)
    wvt_d = nc.declare_dram_parameter("wvt", [C, C], BF16, isOutput=False)
    wot_d = nc.declare_dram_parameter("wot", [C, C], BF16, isOutput=False)
    w1t_d = nc.declare_dram_parameter("w1t", [C, 4 * C], BF16, isOutput=False)
    w2t_d = nc.declare_dram_parameter("w2t", [4 * C, C], BF16, isOutput=False)
    b1p_d = nc.declare_dram_parameter("b1p", [128, 8], F32, isOutput=False)
    b2p_d = nc.declare_dram_parameter("b2p", [128, 2], F32, isOutput=False)
    wbc_d = nc.declare_dram_parameter("wbc", [128, 8], F32, isOutput=False)
    out_d = nc.declare_dram_parameter("out", [NSH, C], F32, isOutput=True)

    with tile.TileContext(nc) as tc, ExitStack() as ctx:
        pool = ctx.enter_context(tc.tile_pool(name="p", bufs=1))
        psmall = ctx.enter_context(tc.tile_pool(name="psm", bufs=1))
        pool2 = ctx.enter_context(tc.tile_pool(name="p2", bufs=4))
        poolT = ctx.enter_context(tc.tile_pool(name="pt", bufs=5))
        poolTM = ctx.enter_context(tc.tile_pool(name="ptm", bufs=2))
        poolPP = ctx.enter_context(tc.tile_pool(name="pp", bufs=9))
        psT = ctx.enter_context(tc.tile_pool(name="psT", bufs=2, space="PSUM"))
        psS = ctx.enter_context(tc.tile_pool(name="psS", bufs=2, space="PSUM"))
        psA = ctx.enter_context(tc.tile_pool(name="psA", bufs=2, space="PSUM"))

        ll_mlp = nc.gpsimd.load_library(library_config.mlp)

        # ---------------- input loads (gather deps first) ----------------
        PGIDX = pool.tile([128, 8, 4, 64], I16)
        nc.sync.dma_start(out=PGIDX[:], in_=pgidx_d[:])
        ident = pool.tile([128, 128], F32)
        nc.sync.dma_start(out=ident[:], in_=ident_d[:])
        NEGB = pool.tile([128, 1], F32)
        nc.vector.memset(NEGB[:], -EXPB)
        X = pool.tile([128, 16, C], F32, tag="X")        # slot later reused by W2
        nc.sync.dma_start(out=X[:], in_=x_d[:].rearrange("(t p) c -> p t c", p=128))
        XOWN = pool.tile([128, NBLK, C], F32)
        nc.sync.dma_start(out=XOWN[:], in_=xo_d[:].rearrange("(t p) c -> p t c", p=128))
        WQ = pool.tile([128, 2, C], BF16)
        nc.sync.dma_start(out=WQ[:], in_=wqt_d[:].rearrange("(u p) c -> p u c", p=128))
        WK = pool.tile([128, 2, C], BF16)
        nc.sync.dma_start(out=WK[:], in_=wkt_d[:].rearrange("(u p) c -> p u c", p=128))
        WV = pool.tile([128, 2, C], BF16)
        nc.sync.dma_start(out=WV[:], in_=wvt_d[:].rearrange("(u p) c -> p u c", p=128))
        WO = pool.tile([128, 2, C], BF16)
        nc.sync.dma_start(out=WO[:], in_=wot_d[:].rearrange("(u p) c -> p u c", p=128))
        B1 = pool.tile([128, 8], F32)
        nc.sync.dma_start(out=B1[:], in_=b1p_d[:])
        B2 = pool.tile([128, 2], F32)
        nc.sync.dma_start(out=B2[:], in_=b2p_d[:])
        WBC = pool.tile([128, 8], F32)
        nc.sync.dma_start(out=WBC[:], in_=wbc_d[:])
        PGPAR = pool.tile([128, 256], F32)
        nc.sync.dma_start(out=PGPAR[:], in_=pgpar_d[:])
        SCIDX = pool.tile([128, NBLK, 2, 128], I16)
        nc.sync.dma_start(out=SCIDX[:], in_=scidx_d[:])
        LM15D = pool.tile([128, NBLK, N], BF16)
        nc.sync.dma_start(out=LM15D[:], in_=lm15_d[:])

        # ------- fast LN: one ACT Square pass, big DVE reduces, ------------
        # ------- per-chunk ACT Identity normalize --------------------------
        ln_state = {}

        def layernorm_act(dst, src, nt, nm, sqtile=None):
            RS1 = psmall.tile([128, nt], F32, tag="RS1", name=f"RS1{nm}")
            RS2 = psmall.tile([128, nt], F32, tag="RS2", name=f"RS2{nm}")
            nc.vector.reduce_sum(RS1[:], src[:], axis=AX.X)
            if sqtile is None:
                sqtile = poolTM.tile([128, nt * C], F32, tag="DUM", name=f"SQ{nm}")
            SQ = sqtile[:].rearrange("p (t c) -> p t c", t=nt)
            nc.scalar.activation(SQ, src[:], AF.Square)
            r = nc.vector.reduce_sum(RS2[:], SQ, axis=AX.X)
            ln_state["last_reduce"] = r
            MU = psmall.tile([128, nt], F32, tag="MU", name=f"MU{nm}")
            nc.vector.tensor_scalar_mul(out=MU[:], in0=RS1[:], scalar1=1.0 / C)
            VAR = psmall.tile([128, nt], F32, tag="VAR", name=f"VAR{nm}")
            nc.vector.tensor_scalar_mul(out=VAR[:], in0=RS2[:], scalar1=1.0 / C)
            MSQ = psmall.tile([128, nt], F32, tag="MSQ", name=f"MSQ{nm}")
            nc.vector.tensor_tensor(out=MSQ[:], in0=MU[:], in1=MU[:], op=ALU.mult)
            nc.vector.tensor_tensor(out=VAR[:], in0=VAR[:], in1=MSQ[:], op=ALU.subtract)
            nc.vector.tensor_scalar_add(out=VAR[:], in0=VAR[:], scalar1=1e-5)
            RSTD = psmall.tile([128, nt], F32, tag="RSTD", name=f"RSTD{nm}")
            nc.vector.reciprocal(RSTD[:], VAR[:])
            nc.scalar.activation(RSTD[:], RSTD[:], AF.Sqrt)
            NMR = psmall.tile([128, nt], F32, tag="NMR", name=f"NMR{nm}")
            nc.vector.tensor_tensor(out=NMR[:], in0=MU[:], in1=RSTD[:], op=ALU.mult)
            nc.vector.tensor_scalar_mul(out=NMR[:], in0=NMR[:], scalar1=-1.0)
            for t in range(nt):
                nc.scalar.activation(dst[:, t, :], src[:, t, :], AF.Identity,
                                     scale=RSTD[:, t:t + 1], bias=NMR[:, t:t + 1])

        X2 = pool.tile([128, 16 * C], F32, tag="XT", name="X2")
        XLN = pool.tile([128, 16, C], F32, tag="GSG", name="XLN")
        layernorm_act(XLN, X, 16, "a", sqtile=X2)
        XLNO = pool.tile([128, NBLK, C], F32)
        layernorm_act(XLNO, XOWN, NBLK, "b")
        ln_last = ln_state["last_reduce"]
        # preload the Exp ACT table before the softmax phase needs it
        EWARM = psmall.tile([128, 1], F32, tag="EWARM", name="EWARM")
        nc.scalar.activation(EWARM[:], NEGB[:], AF.Exp)

        # ---------------- transposes (evacs on ACT) ----------------
        XT = pool.tile([128, 2, N], BF16, tag="XT")
        for t in range(16):
            for u in range(2):
                tp = psT.tile([128, 128], F32, tag="tp", name="tp")
                nc.tensor.transpose(out=tp[:], in_=XLN[:, t, u * 128:(u + 1) * 128], identity=ident[:])
                nc.scalar.activation(XT[:, u, t * 128:(t + 1) * 128], tp[:], AF.Copy)
        XQT = pool.tile([128, 2, NSH], BF16)
        XOT = pool.tile([128, 2, NSH], F32)
        for t in range(NBLK):
            for u in range(2):
                tp = psT.tile([128, 128], F32, tag="tp", name="tp")
                nc.tensor.transpose(out=tp[:], in_=XLNO[:, t, u * 128:(u + 1) * 128], identity=ident[:])
                nc.scalar.activation(XQT[:, u, t * 128:(t + 1) * 128], tp[:], AF.Copy)
                tp2 = psT.tile([128, 128], F32, tag="tp", name="tp2")
                nc.tensor.transpose(out=tp2[:], in_=XOWN[:, t, u * 128:(u + 1) * 128], identity=ident[:])
                nc.vector.tensor_copy(XOT[:, u, t * 128:(t + 1) * 128], tp2[:])

        # ---------------- K^T, V^T, Q^T (evacs on ACT) ----------------
        KT = pool.tile([128, 2, N], BF16, tag="KT")
        VTB = pool.tile([128, 2, N], BF16, tag="VTB")
        for ch in range(2):
            for jc in range(4):
                kp = psS.tile([128, 1024], F32, tag="ps", name="kp")
                for u in range(2):
                    nc.tensor.matmul(
                        kp[:, :512], WK[:, u, ch * 128:(ch + 1) * 128],
                        XT[:, u, jc * 512:(jc + 1) * 512],
                        start=(u == 0), stop=(u == 1))
                for u in range(2):
                    nc.tensor.matmul(
                        kp[:, 512:], WV[:, u, ch * 128:(ch + 1) * 128],
                        XT[:, u, jc * 512:(jc + 1) * 512],
                        start=(u == 0), stop=(u == 1))
                nc.scalar.activation(KT[:, ch, jc * 512:(jc + 1) * 512], kp[:, :512], AF.Copy)
                nc.scalar.activation(VTB[:, ch, jc * 512:(jc + 1) * 512], kp[:, 512:], AF.Copy)
        QT = pool.tile([128, 2, NSH], BF16)
        for ch in range(2):
            qp = psS.tile([128, 1024], F32, tag="ps", name="qp")
            for u in range(2):
                nc.tensor.matmul(
                    qp[:, :NSH], WQ[:, u, ch * 128:(ch + 1) * 128],
                    XQT[:, u, :],
                    start=(u == 0), stop=(u == 1))
            nc.scalar.activation(QT[:, ch, :], qp[:, :NSH], AF.Copy)

        VR = pool.tile([128, 16, C], BF16, tag="VR")
        for ch in range(2):
            nc.sync.dma_start(out=VR[:, :, ch * 128:(ch + 1) * 128], in_=VTB[:, ch, :], transpose=True)

        # ----- pair gathers (4-queue) + R2 reduces, alternating with -------
        # ----- phase-A of the softmax pipeline (scores + lm15 evac) --------
        PMG = pool.tile([128, 256], F32)                 # [b, i]; /32 folded into wbc
        PMGH = pool.tile([128, 256], BF16)
        D = pool.tile([128, NBLK * H], F32)
        RD = pool.tile([128, NBLK * H], F32)
        gathers = []
        PPs = []
        r2_lasts = []
        tt1_lasts = []

        def emit_gather_chunk(cc):
            c, qh = cc // 2, cc % 2
            GPc = pool2.tile([128, 16, 64], F32, tag="GP", name=f"GP{cc}")
            for q2 in range(2):
                q = qh * 2 + q2
                g = nc.gpsimd.dma_gather(
                    out_ap=GPc[:, q2 * 8:(q2 + 1) * 8, :],
                    in_ap=pairs_d[c * 32768:(c + 1) * 32768, :],
                    idxs_ap=PGIDX[:, c, q, :],
                    num_idxs=1024,
                    num_idxs_reg=1024,
                    elem_size=64,
                    queue_num=len(gathers) % 4,
                )
                add_dep_helper(g.ins, ll_mlp.ins, reason="gather needs mlp lib")
                gathers.append(g)
            R2 = psmall.tile([128, 16, 2], F32, tag="R2", name=f"R2{cc}")
            r = nc.vector.reduce_sum(R2[:], GPc[:].rearrange("p i (t f) -> p i t f", t=2), axis=AX.X)
            if cc < 3:
                add_dep_helper(r.ins, ln_last.ins, reason="R2 after LN reduces on DVE")
            elif len(tt1_lasts) >= 2 * (cc - 2):
                add_dep_helper(r.ins, tt1_lasts[2 * (cc - 2) - 1].ins,
                               reason="R2 paced behind TT1 pre-runs")
            sl = slice(cc * 16, (cc + 1) * 16)
            nc.vector.tensor_tensor(out=PMG[:, sl], in0=R2[:, :, 1], in1=R2[:, :, 0], op=ALU.subtract)
            nc.vector.tensor_tensor(out=PMG[:, sl], in0=PMG[:, sl], in1=PGPAR[:, sl], op=ALU.mult)
            r2_lasts.append(nc.vector.tensor_tensor(out=PMGH[:, sl], in0=PMG[:, sl], in1=R2[:, :, 0], op=ALU.add))

        def emit_phase_a(hx):
            blk, h = hx // H, hx % H
            r0 = (h % 4) * 32
            qs = QT[r0:r0 + 32, h // 4, blk * 128:(blk + 1) * 128]
            PP = poolPP.tile([128, N], BF16, tag="PP", name=f"PP{hx}")
            for jc2 in range(2):
                ps = psS.tile([128, 1024], F32, tag="ps", name=f"ps{hx}_{jc2}")
                for jc in range(2):
                    j0 = (jc2 * 2 + jc) * 512
                    nc.tensor.matmul(
                        ps[:, jc * 512:(jc + 1) * 512], qs,
                        KT[r0:r0 + 32, h // 4, j0:j0 + 512],
                        start=True, stop=True, tile_position=(r0, 0))
                t1 = nc.vector.tensor_tensor(
                    out=PP[:, jc2 * 1024:(jc2 + 1) * 1024],
                    in0=ps[:], in1=LM15D[:, blk, jc2 * 1024:(jc2 + 1) * 1024],
                    op=ALU.add)
                tt1_lasts.append(t1)
            PPs.append(PP)

        for cc in range(16):
            emit_gather_chunk(cc)
            if cc < 9:
                emit_phase_a(cc)

        chain_depth = int(os.environ.get("KCHAIN", "2"))
        if chain_depth > 0:
            stride = 4 * chain_depth       # per-queue ring reclaim chain
            for n in range(stride, len(gathers)):
                add_dep_helper(gathers[n].ins, gathers[n - stride].ins, sync=True,
                               reason="swdge ring reclaim chain")

        # ---------------- pm transpose (xbar) + dense scatter --------------
        pmTT = pool.tile([128, NBLK, 128], BF16)
        nc.sync.dma_start(out=pmTT[:], in_=PMGH[:], transpose=True)

        ll_ls = nc.gpsimd.load_library(library_config.local_scatter)
        for g in gathers:
            add_dep_helper(ll_ls.ins, g.ins, reason="lib switch after gathers")

        PMD = pool.tile([128, NBLK, N], BF16)
        for blk in range(NBLK):
            for half in range(2):
                ls = nc.gpsimd.local_scatter(
                    out_ap=PMD[:, blk, half * 1024:(half + 1) * 1024],
                    data_ap=pmTT[:, blk, :],
                    idxs_ap=SCIDX[:, blk, half, :],
                    channels=128,
                    num_elems=1024,
                    num_idxs=128,
                )
                add_dep_helper(ls.ins, ll_ls.ins, reason="scatter needs ls lib")

        # ------------- phase-B: bias, exp, normalize, transpose, AV --------
        ATT = pool.tile([128, 2, NSH], BF16, tag="ATT")
        PTmap = {}

        def emit_phase_b(hx):
            blk, h = hx // H, hx % H
            PP = PPs[hx]
            TMP = poolTM.tile([128, N], BF16, tag="TMP", name=f"TMP{hx}")
            nc.vector.tensor_scalar_mul(out=TMP[:], in0=PMD[:, blk, :],
                                        scalar1=WBC[:, h:h + 1])
            nc.vector.tensor_tensor(out=PP[:], in0=PP[:], in1=TMP[:], op=ALU.add)
            nc.scalar.activation(PP[:], PP[:], AF.Exp, bias=NEGB[:],
                                 accum_out=D[:, hx:hx + 1])
            nc.vector.reciprocal(RD[:, hx:hx + 1], D[:, hx:hx + 1])
            nc.vector.tensor_scalar_mul(out=PP[:], in0=PP[:], scalar1=RD[:, hx:hx + 1])
            PT = poolT.tile([128, 16, 128], BF16, tag="PT", name=f"PT{hx}")
            nc.sync.dma_start(out=PT[:], in_=PP[:], transpose=True)
            PTmap[hx] = PT
            if h % 4 == 3:
                hg = h // 4
                av = psA.tile([128, 128], F32, tag="av", name="av")
                for k in range(4):
                    for jh in range(16):
                        nc.tensor.matmul(
                            av[k * 32:(k + 1) * 32, :],
                            VR[:, jh, hg * 128 + k * 32:hg * 128 + (k + 1) * 32],
                            PTmap[blk * H + hg * 4 + k][:, jh, :],
                            start=(jh == 0), stop=(jh == 15),
                            tile_position=(0, k * 32))
                if hg % 2 == 0:
                    nc.vector.tensor_copy(
                        ATT[:, hg, blk * 128:(blk + 1) * 128], av[:])
                else:
                    nc.scalar.activation(
                        ATT[:, hg, blk * 128:(blk + 1) * 128], av[:], AF.Copy)

        # -------- per-block tail: outproj, LN2, MLP, store -----------------
        W1 = pool.tile([128, 2, 4 * C], BF16, tag="GSG", name="W1")
        nc.sync.dma_start(out=W1[:], in_=w1t_d[:].rearrange("(u p) c -> p u c", p=128))
        W2 = pool.tile([128, 8, C], BF16, tag="X", name="W2")
        nc.sync.dma_start(out=W2[:], in_=w2t_d[:].rearrange("(u p) c -> p u c", p=128))

        Y1T = pool.tile([128, 2, NSH], F32, tag="Y1T")
        Y1 = pool.tile([128, NBLK, C], F32, tag="Y1")
        H2 = pool.tile([128, NBLK, C], F32, tag="H2")
        H2T = pool.tile([128, 2, NSH], BF16, tag="H2T")
        M1 = pool.tile([128, 8, NSH], BF16, tag="XT", name="M1")
        YT = pool.tile([128, 2, NSH], F32, tag="YT")
        OUT = pool.tile([128, NBLK, C], F32, tag="OUT")

        def emit_tail_block(blk):
            bs = slice(blk * 128, (blk + 1) * 128)
            # out-projection + residual (transposed space)
            for ch in range(2):
                op_ = psS.tile([128, 1024], F32, tag="ps", name="op")
                for u in range(2):
                    nc.tensor.matmul(
                        op_[:, :128], WO[:, u, ch * 128:(ch + 1) * 128],
                        ATT[:, u, bs],
                        start=(u == 0), stop=(u == 1))
                nc.vector.tensor_tensor(out=Y1T[:, ch, bs], in0=op_[:, :128],
                                        in1=XOT[:, ch, bs], op=ALU.add)
                tp = psT.tile([128, 128], F32, tag="tp", name="tp")
                nc.tensor.transpose(out=tp[:], in_=Y1T[:, ch, bs], identity=ident[:])
                nc.vector.tensor_copy(Y1[:, blk, ch * 128:(ch + 1) * 128], tp[:])
            # LN2 for this block
            layernorm_act(H2[:, blk:blk + 1, :], Y1[:, blk:blk + 1, :], 1, f"c{blk}")
            for ch in range(2):
                tp = psT.tile([128, 128], F32, tag="tp", name="tp")
                nc.tensor.transpose(out=tp[:], in_=H2[:, blk, ch * 128:(ch + 1) * 128], identity=ident[:])
                nc.scalar.activation(H2T[:, ch, bs], tp[:], AF.Copy)
            # MLP for this block
            for mc in range(8):
                mp = psS.tile([128, 1024], F32, tag="ps", name="mp")
                for u in range(2):
                    nc.tensor.matmul(
                        mp[:, :128], W1[:, u, mc * 128:(mc + 1) * 128],
                        H2T[:, u, bs],
                        start=(u == 0), stop=(u == 1))
                TM = psmall.tile([128, 128], BF16, tag="TM", name="TM")
                nc.vector.tensor_scalar_add(out=TM[:], in0=mp[:, :128], scalar1=B1[:, mc:mc + 1])
                nc.scalar.activation(M1[:, mc, bs], mp[:, :128], AF.Sigmoid, bias=B1[:, mc:mc + 1])
                nc.vector.tensor_tensor(out=M1[:, mc, bs], in0=M1[:, mc, bs], in1=TM[:], op=ALU.mult)
            for ch in range(2):
                yp = psS.tile([128, 1024], F32, tag="ps", name="yp")
                for mc in range(8):
                    nc.tensor.matmul(
                        yp[:, :128], W2[:, mc, ch * 128:(ch + 1) * 128],
                        M1[:, mc, bs],
                        start=(mc == 0), stop=(mc == 7))
                nc.vector.tensor_scalar_add(out=YT[:, ch, bs], in0=yp[:, :128], scalar1=B2[:, ch:ch + 1])
                nc.vector.tensor_tensor(out=YT[:, ch, bs], in0=YT[:, ch, bs], in1=Y1T[:, ch, bs], op=ALU.add)
                tp = psT.tile([128, 128], F32, tag="tp", name="tp")
                nc.tensor.transpose(out=tp[:], in_=YT[:, ch, bs], identity=ident[:])
                nc.vector.tensor_copy(OUT[:, blk, ch * 128:(ch + 1) * 128], tp[:])
        # late phase-A (PP slots freed by early phase-B transposes), with the
        # blk0 tail interleaved so it overlaps blk1's softmax phase
        for k in range(16):
            emit_phase_b(k)
            if 0 <= k < 7 and 9 + k < 16:
                emit_phase_a(9 + k)
            if k == 7:
                emit_tail_block(0)
        emit_tail_block(1)

        nc.sync.dma_start(out=out_d[:].rearrange("(t p) c -> p t c", p=128), in_=OUT[:])

    nc.compile()
    return nc


# ======================= host side =======================

def _wrap16(flat):
    """dma_gather index layout: idx k at [k%16, k//16], replicated x8 groups."""
    n = flat.shape[0]
    arr = flat.reshape(n // 16, 16).T.astype(np.int16)
    return np.tile(arr, (8, 1))


def _host_prep(core, x, pair_emb, block_index, Wq, Wk, Wv, Wb, Wout, W1, b1, W2, b2):
    i0 = core * NSH
    idx = np.asarray(block_index[i0:i0 + NSH]).astype(np.int64)   # [256, 128]

    pgidx = np.zeros((128, 8, 4, 64), np.int16)
    for c in range(8):
        sub = idx[c * 32:(c + 1) * 32]                            # [32, 128]
        flat = (np.arange(32)[:, None] * 1024 + sub // 2).reshape(-1)
        for q in range(4):
            pgidx[:, c, q, :] = _wrap16(flat[q * 1024:(q + 1) * 1024])
    pgpar = np.ascontiguousarray((idx % 2).astype(np.float32).T)  # [b, i]

    lm15 = np.zeros((128, NBLK, N), ml_dtypes.bfloat16)
    scidx = np.zeros((128, NBLK, 2, 128), np.int16)
    for blk in range(NBLK):
        sub = idx[blk * 128:(blk + 1) * 128]                      # [128 i, 128 b]
        for i in range(128):
            row = sub[i]
            uniq, first_pos, counts = np.unique(row, return_index=True, return_counts=True)
            lm15[i, blk, uniq] = (np.log(counts.astype(np.float64)) + EXPB).astype(ml_dtypes.bfloat16)
            scrow = np.full(128, -1, np.int64)
            scrow[first_pos] = uniq
            for half in range(2):
                sc = np.where((scrow >= 1024 * half) & (scrow < 1024 * (half + 1)),
                              scrow - 1024 * half, -1)
                scidx[i, blk, half, :] = sc.astype(np.int16)

    scale = 1.0 / math.sqrt(DH)
    fp = np.float32
    bf = ml_dtypes.bfloat16
    feeds = {
        "x": np.ascontiguousarray(x, fp),
        "xown": np.ascontiguousarray(np.asarray(x, fp)[i0:i0 + NSH]),
        "pairs": np.ascontiguousarray(
            np.asarray(pair_emb[i0:i0 + NSH], fp).reshape(NSH * 1024, 64)),
        "pgidx": pgidx, "pgpar": pgpar, "scidx": scidx, "lm15": lm15,
        "ident": np.eye(128, dtype=fp),
        "wqt": np.ascontiguousarray(np.asarray(Wq, fp).T * scale).astype(bf),
        "wkt": np.ascontiguousarray(np.asarray(Wk, fp).T).astype(bf),
        "wvt": np.ascontiguousarray(np.asarray(Wv, fp).T).astype(bf),
        "wot": np.ascontiguousarray(np.asarray(Wout, fp).T).astype(bf),
        "w1t": np.ascontiguousarray(np.asarray(W1, fp).T).astype(bf),
        "w2t": np.ascontiguousarray(np.asarray(W2, fp).T).astype(bf),
        "b1p": np.ascontiguousarray(np.asarray(b1, fp).reshape(8, 128).T),
        "b2p": np.ascontiguousarray(np.asarray(b2, fp).reshape(2, 128).T),
        "wbc": np.tile(np.asarray(Wb, fp).reshape(1, 8) / CP, (128, 1)),
    }
    return feeds


_NC = None


def kernel(**inputs):
    global _NC
    from concourse.bass_utils import run_bass_kernel_spmd
    if _NC is None:
        _NC = build_nc()
    in_maps = [_host_prep(core, **inputs) for core in range(NCORES)]
    r = run_bass_kernel_spmd(_NC, in_maps, core_ids=list(range(NCORES)))
    out = np.concatenate([np.asarray(r.results[i]["out"]).reshape(NSH, C)
                          for i in range(NCORES)], axis=0)
    return out.astype(np.float32)


# revision 25
# speedup vs baseline: 1.1410x; 1.1410x over previous
# kernel.py — AtomTransformerBlock on 8 TRN2 NeuronCores (SPMD, no collectives).
#
# Sharding: N_atom rows across 8 cores (256 rows each); x + weights replicated
# (each core recomputes LN(x), K, V for all 2048 rows). pair_emb sharded by
# first axis. All index-derived masks are precomputed on the host (pure index
# preprocessing); all tensor math happens on device.
#
# Design — dense softmax (no score DRAM roundtrip):
#   pair-bias dma_gather (256B elements, 4 SWDGE queues in parallel; the
#   desc-gen ucode runs on Q7 core-pair `queue_num`, so 4 queues give ~4x
#   descriptor throughput) -> per-chunk channel reduce on DVE -> PMG[b, i] ->
#   xbar transpose -> local_scatter of pm values into dense-j space
#   (PMD[i, j], zeros elsewhere). Host feeds the dense log-multiplicity map
#   LM15D[i, j] = log(count)+EXPB at gathered j, 0 elsewhere. Scores stay
#   dense: S'[i,h,j] = q.k + lm15 (fused into the psum evac) + pm*wb[h], and
#   P = exp(S' - EXPB) kills non-gathered j. ACT accum_out yields the softmax
#   denominator during the exp; normalize is a per-partition tensor_scalar;
#   AV contracts dense P^T (DMA transpose) against dense V on PE with
#   col-tiled (M=32) matmuls, 4 heads per psum tile.
#   LN runs on ACT (per-chunk Copy/Square with accum_out reduces + Identity
#   scale/bias normalize) so the DVE stays free for the pair reduces, whose
#   in-order stream paces the gather phase. LN's few tiny DVE stats ops are
#   emitted between gather chunks 5 and 6 so they neither stall the gathers
#   nor wait behind the full reduce chain. The per-block tail (out-proj, LN2,
#   MLP) for block 0 is emitted mid-softmax so it overlaps block 1's phase.
import math
import os
import sys

import numpy as np

sys.path.insert(0, "/opt/trn_rl_repo")

import ml_dtypes
from contextlib import ExitStack

import concourse.bass as bass
import concourse.mybir as mybir
import concourse.tile as tile
from concourse import bacc, library_config
from concourse.tile import add_dep_helper

N, C, H, DH, CP, B = 2048, 256, 8, 32, 32, 128
NCORES = 8
NSH = N // NCORES          # 256 rows per core
NBLK = NSH // 128          # 2 i-blocks per core
F32 = mybir.dt.float32
BF16 = mybir.dt.bfloat16
I16 = mybir.dt.int16
AX = mybir.AxisListType
ALU = mybir.AluOpType
AF = mybir.ActivationFunctionType

EXPB = 12.0                # dense-softmax exclusion offset


def build_nc():
    nc = bacc.Bacc(None, target_bir_lowering=False, debug=True, num_swdge_queues=4)

    x_d = nc.declare_dram_parameter("x", [N, C], F32, isOutput=False)
    xo_d = nc.declare_dram_parameter("xown", [NSH, C], F32, isOutput=False)
    pairs_d = nc.declare_dram_parameter("pairs", [NSH * 1024, 64], F32, isOutput=False)
    pgidx_d = nc.declare_dram_parameter("pgidx", [128, 8, 4, 64], I16, isOutput=False)
    pgpar_d = nc.declare_dram_parameter("pgpar", [128, 256], F32, isOutput=False)
    scidx_d = nc.declare_dram_parameter("scidx", [128, NBLK, 2, 128], I16, isOutput=False)
    lm15_d = nc.declare_dram_parameter("lm15", [128, NBLK, N], BF16, isOutput=False)
    ident_d = nc.declare_dram_parameter("ident", [128, 128], F32, isOutput=False)
    wqt_d = nc.declare_dram_parameter("wqt", [C, C], BF16, isOutput=False)
    wkt_d = nc.declare_dram_parameter("wkt", [C, C], BF16, isOutput=False)
    wvt_d = nc.declare_dram_parameter("wvt", [C, C], BF16, isOutput=False)
    wot_d = nc.declare_dram_parameter("wot", [C, C], BF16, isOutput=False)
    w1t_d = nc.declare_dram_parameter("w1t", [C, 4 * C], BF16, isOutput=False)
    w2t_d = nc.declare_dram_parameter("w2t", [4 * C, C], BF16, isOutput=False)
    b1p_d = nc.declare_dram_parameter("b1p", [128, 8], F32, isOutput=False)
    b2p_d = nc.declare_dram_parameter("b2p", [128, 2], F32, isOutput=False)
    wbc_d = nc.declare_dram_parameter("wbc", [128, 8], F32, isOutput=False)
    out_d = nc.declare_dram_parameter("out", [NSH, C], F32, isOutput=True)

    with tile.TileContext(nc) as tc, ExitStack() as ctx:
        pool = ctx.enter_context(tc.tile_pool(name="p", bufs=1))
        psmall = ctx.enter_context(tc.tile_pool(name="psm", bufs=1))
        pool2 = ctx.enter_context(tc.tile_pool(name="p2", bufs=4))
        poolT = ctx.enter_context(tc.tile_pool(name="pt", bufs=5))
        poolTM = ctx.enter_context(tc.tile_pool(name="ptm", bufs=2))
        poolPP = ctx.enter_context(tc.tile_pool(name="pp", bufs=9))
        psT = ctx.enter_context(tc.tile_pool(name="psT", bufs=2, space="PSUM"))
        psS = ctx.enter_context(tc.tile_pool(name="psS", bufs=2, space="PSUM"))
        psA = ctx.enter_context(tc.tile_pool(name="psA", bufs=2, space="PSUM"))

        ll_mlp = nc.gpsimd.load_library(library_config.mlp)

        # ---------------- input loads (gather deps first) ----------------
        PGIDX = pool.tile([128, 8, 4, 64], I16)
        nc.sync.dma_start(out=PGIDX[:], in_=pgidx_d[:])
        ident = pool.tile([128, 128], F32)
        nc.sync.dma_start(out=ident[:], in_=ident_d[:])
        NEGB = pool.tile([128, 1], F32)
        nc.vector.memset(NEGB[:], -EXPB)
        X = pool.tile([128, 16, C], F32, tag="X")        # slot later reused by W2
        nc.sync.dma_start(out=X[:], in_=x_d[:].rearrange("(t p) c -> p t c", p=128))
        XOWN = pool.tile([128, NBLK, C], F32)
        nc.sync.dma_start(out=XOWN[:], in_=xo_d[:].rearrange("(t p) c -> p t c", p=128))
        WQ = pool.tile([128, 2, C], BF16)
        nc.sync.dma_start(out=WQ[:], in_=wqt_d[:].rearrange("(u p) c -> p u c", p=128))
        WK = pool.tile([128, 2, C], BF16)
        nc.sync.dma_start(out=WK[:], in_=wkt_d[:].rearrange("(u p) c -> p u c", p=128))
        WV = pool.tile([128, 2, C], BF16)
        nc.sync.dma_start(out=WV[:], in_=wvt_d[:].rearrange("(u p) c -> p u c", p=128))
        WO = pool.tile([128, 2, C], BF16)
        nc.sync.dma_start(out=WO[:], in_=wot_d[:].rearrange("(u p) c -> p u c", p=128))
        B1 = pool.tile([128, 8], F32)
        nc.sync.dma_start(out=B1[:], in_=b1p_d[:])
        B2 = pool.tile([128, 2], F32)
        nc.sync.dma_start(out=B2[:], in_=b2p_d[:])
        WBC = pool.tile([128, 8], F32)
        nc.sync.dma_start(out=WBC[:], in_=wbc_d[:])
        PGPAR = pool.tile([128, 256], F32)
        nc.sync.dma_start(out=PGPAR[:], in_=pgpar_d[:])
        SCIDX = pool.tile([128, NBLK, 2, 128], I16)
        nc.sync.dma_start(out=SCIDX[:], in_=scidx_d[:])
        LM15D = pool.tile([128, NBLK, N], BF16)
        nc.sync.dma_start(out=LM15D[:], in_=lm15_d[:])

        # ------- LN on ACT: accum_out reduces + Identity normalize ---------
        # DVE does only a handful of [128, nt] stats ops per LN; the caller
        # controls where those land in the DVE stream.
        def ln_reduce(src, nt, nm):
            RS1 = psmall.tile([128, nt], F32, tag=f"RS1{nm}", name=f"RS1{nm}")
            RS2 = psmall.tile([128, nt], F32, tag=f"RS2{nm}", name=f"RS2{nm}")
            for t in range(nt):
                DUM = poolTM.tile([128, C], F32, tag="DUM", name=f"DU{nm}{t}")
                nc.scalar.activation(DUM[:], src[:, t, :], AF.Copy,
                                     accum_out=RS1[:, t:t + 1])
                DUM2 = poolTM.tile([128, C], F32, tag="DUM", name=f"DV{nm}{t}")
                nc.scalar.activation(DUM2[:], src[:, t, :], AF.Square,
                                     accum_out=RS2[:, t:t + 1])
            return RS1, RS2

        def ln_stats(RS1, RS2, nt, nm):
            MU = psmall.tile([128, nt], F32, tag=f"MU{nm}", name=f"MU{nm}")
            nc.vector.tensor_scalar_mul(out=MU[:], in0=RS1[:], scalar1=1.0 / C)
            VAR = psmall.tile([128, nt], F32, tag=f"VAR{nm}", name=f"VAR{nm}")
            nc.vector.tensor_scalar_mul(out=VAR[:], in0=RS2[:], scalar1=1.0 / C)
            MSQ = psmall.tile([128, nt], F32, tag=f"MSQ{nm}", name=f"MSQ{nm}")
            nc.vector.tensor_tensor(out=MSQ[:], in0=MU[:], in1=MU[:], op=ALU.mult)
            nc.vector.tensor_tensor(out=VAR[:], in0=VAR[:], in1=MSQ[:], op=ALU.subtract)
            nc.vector.tensor_scalar_add(out=VAR[:], in0=VAR[:], scalar1=1e-5)
            RSTD = psmall.tile([128, nt], F32, tag=f"RSTD{nm}", name=f"RSTD{nm}")
            nc.vector.reciprocal(RSTD[:], VAR[:])
            nc.scalar.activation(RSTD[:], RSTD[:], AF.Sqrt)
            NMR = psmall.tile([128, nt], F32, tag=f"NMR{nm}", name=f"NMR{nm}")
            nc.vector.tensor_tensor(out=NMR[:], in0=MU[:], in1=RSTD[:], op=ALU.mult)
            nc.vector.tensor_scalar_mul(out=NMR[:], in0=NMR[:], scalar1=-1.0)
            return RSTD, NMR

        def ln_normalize(dst, src, RSTD, NMR, nt):
            for t in range(nt):
                nc.scalar.activation(dst[:, t, :], src[:, t, :], AF.Identity,
                                     scale=RSTD[:, t:t + 1], bias=NMR[:, t:t + 1])

        # ACT starts the LN reduces immediately (they only need X/XOWN).
        RS1a, RS2a = ln_reduce(X, 16, "a")
        RS1b, RS2b = ln_reduce(XOWN, NBLK, "b")

        # ----- pair gathers (4-queue) + R2 reduces on DVE ------------------
        PMG = pool.tile([128, 256], F32)                 # [b, i]; /32 folded into wbc
        PMGH = pool.tile([128, 256], BF16)
        gathers = []

        def emit_gather_chunk(cc):
            c, qh = cc // 2, cc % 2
            GPc = pool2.tile([128, 16, 64], F32, tag="GP", name=f"GP{cc}")
            for q2 in range(2):
                q = qh * 2 + q2
                g = nc.gpsimd.dma_gather(
                    out_ap=GPc[:, q2 * 8:(q2 + 1) * 8, :],
                    in_ap=pairs_d[c * 32768:(c + 1) * 32768, :],
                    idxs_ap=PGIDX[:, c, q, :],
                    num_idxs=1024,
                    num_idxs_reg=1024,
                    elem_size=64,
                    queue_num=len(gathers) % 4,
                )
                add_dep_helper(g.ins, ll_mlp.ins, reason="gather needs mlp lib")
                gathers.append(g)
            R2 = psmall.tile([128, 16, 2], F32, tag="R2", name=f"R2{cc}")
            nc.vector.reduce_sum(R2[:], GPc[:].rearrange("p i (t f) -> p i t f", t=2), axis=AX.X)
            sl = slice(cc * 16, (cc + 1) * 16)
            nc.vector.tensor_tensor(out=PMG[:, sl], in0=R2[:, :, 1], in1=R2[:, :, 0], op=ALU.subtract)
            nc.vector.tensor_tensor(out=PMG[:, sl], in0=PMG[:, sl], in1=PGPAR[:, sl], op=ALU.mult)
            nc.vector.tensor_tensor(out=PMGH[:, sl], in0=PMG[:, sl], in1=R2[:, :, 0], op=ALU.add)

        for cc in range(6):
            emit_gather_chunk(cc)

        # LN stats (tiny DVE ops) woven in here: ready once the ACT reduces
        # finish (~mid-gather), ordered after R2(5) so they never stall the
        # gather/R2 pacing at the head of the DVE stream.
        RSTDa, NMRa = ln_stats(RS1a, RS2a, 16, "a")
        RSTDb, NMRb = ln_stats(RS1b, RS2b, NBLK, "b")
        XLN = pool.tile([128, 16, C], F32, tag="GSG", name="XLN")
        ln_normalize(XLN, X, RSTDa, NMRa, 16)
        XLNO = pool.tile([128, NBLK, C], F32)
        ln_normalize(XLNO, XOWN, RSTDb, NMRb, NBLK)
        # preload the Exp ACT table before the softmax phase needs it
        EWARM = psmall.tile([128, 1], F32, tag="EWARM", name="EWARM")
        nc.scalar.activation(EWARM[:], NEGB[:], AF.Exp)

        for cc in range(6, 16):
            emit_gather_chunk(cc)

        chain_depth = int(os.environ.get("KCHAIN", "2"))
        if chain_depth > 0:
            stride = 4 * chain_depth       # per-queue ring reclaim chain
            for n in range(stride, len(gathers)):
                add_dep_helper(gathers[n].ins, gathers[n - stride].ins, sync=True,
                               reason="swdge ring reclaim chain")

        # ---------------- transposes (evacs on ACT) ----------------
        XT = pool.tile([128, 2, N], BF16, tag="XT")
        for t in range(16):
            for u in range(2):
                tp = psT.tile([128, 128], F32, tag="tp", name="tp")
                nc.tensor.transpose(out=tp[:], in_=XLN[:, t, u * 128:(u + 1) * 128], identity=ident[:])
                nc.scalar.activation(XT[:, u, t * 128:(t + 1) * 128], tp[:], AF.Copy)
        XQT = pool.tile([128, 2, NSH], BF16)
        XOT = pool.tile([128, 2, NSH], F32)
        for t in range(NBLK):
            for u in range(2):
                tp = psT.tile([128, 128], F32, tag="tp", name="tp")
                nc.tensor.transpose(out=tp[:], in_=XLNO[:, t, u * 128:(u + 1) * 128], identity=ident[:])
                nc.scalar.activation(XQT[:, u, t * 128:(t + 1) * 128], tp[:], AF.Copy)
                tp2 = psT.tile([128, 128], F32, tag="tp", name="tp2")
                nc.tensor.transpose(out=tp2[:], in_=XOWN[:, t, u * 128:(u + 1) * 128], identity=ident[:])
                nc.scalar.activation(XOT[:, u, t * 128:(t + 1) * 128], tp2[:], AF.Copy)

        # ---------------- K^T, V^T, Q^T (evacs on ACT) ----------------
        KT = pool.tile([128, 2, N], BF16, tag="KT")
        VTB = pool.tile([128, 2, N], BF16, tag="VTB")
        for ch in range(2):
            for jc in range(4):
                kp = psS.tile([128, 1024], F32, tag="ps", name="kp")
                for u in range(2):
                    nc.tensor.matmul(
                        kp[:, :512], WK[:, u, ch * 128:(ch + 1) * 128],
                        XT[:, u, jc * 512:(jc + 1) * 512],
                        start=(u == 0), stop=(u == 1))
                for u in range(2):
                    nc.tensor.matmul(
                        kp[:, 512:], WV[:, u, ch * 128:(ch + 1) * 128],
                        XT[:, u, jc * 512:(jc + 1) * 512],
                        start=(u == 0), stop=(u == 1))
                nc.scalar.activation(KT[:, ch, jc * 512:(jc + 1) * 512], kp[:, :512], AF.Copy)
                nc.scalar.activation(VTB[:, ch, jc * 512:(jc + 1) * 512], kp[:, 512:], AF.Copy)
        QT = pool.tile([128, 2, NSH], BF16)
        for ch in range(2):
            qp = psS.tile([128, 1024], F32, tag="ps", name="qp")
            for u in range(2):
                nc.tensor.matmul(
                    qp[:, :NSH], WQ[:, u, ch * 128:(ch + 1) * 128],
                    XQT[:, u, :],
                    start=(u == 0), stop=(u == 1))
            nc.scalar.activation(QT[:, ch, :], qp[:, :NSH], AF.Copy)

        VR = pool.tile([128, 16, C], BF16, tag="VR")
        for ch in range(2):
            nc.sync.dma_start(out=VR[:, :, ch * 128:(ch + 1) * 128], in_=VTB[:, ch, :], transpose=True)

        # ---------------- pm transpose (xbar) + dense scatter --------------
        pmTT = pool.tile([128, NBLK, 128], BF16)
        nc.sync.dma_start(out=pmTT[:], in_=PMGH[:], transpose=True)

        ll_ls = nc.gpsimd.load_library(library_config.local_scatter)
        for g in gathers:
            add_dep_helper(ll_ls.ins, g.ins, reason="lib switch after gathers")

        PMD = pool.tile([128, NBLK, N], BF16)
        for blk in range(NBLK):
            for half in range(2):
                ls = nc.gpsimd.local_scatter(
                    out_ap=PMD[:, blk, half * 1024:(half + 1) * 1024],
                    data_ap=pmTT[:, blk, :],
                    idxs_ap=SCIDX[:, blk, half, :],
                    channels=128,
                    num_elems=1024,
                    num_idxs=128,
                )
                add_dep_helper(ls.ins, ll_ls.ins, reason="scatter needs ls lib")

        # ---- softmax pipeline state ----
        D = pool.tile([128, NBLK * H], F32)
        RD = pool.tile([128, NBLK * H], F32)
        PPs = {}
        PTmap = {}
        ATT = pool.tile([128, 2, NSH], BF16, tag="ATT")

        def emit_phase_a(hx):
            blk, h = hx // H, hx % H
            r0 = (h % 4) * 32
            qs = QT[r0:r0 + 32, h // 4, blk * 128:(blk + 1) * 128]
            PP = poolPP.tile([128, N], BF16, tag="PP", name=f"PP{hx}")
            for jc2 in range(2):
                ps = psS.tile([128, 1024], F32, tag="ps", name=f"ps{hx}_{jc2}")
                for jc in range(2):
                    j0 = (jc2 * 2 + jc) * 512
                    nc.tensor.matmul(
                        ps[:, jc * 512:(jc + 1) * 512], qs,
                        KT[r0:r0 + 32, h // 4, j0:j0 + 512],
                        start=True, stop=True, tile_position=(r0, 0))
                nc.vector.tensor_tensor(
                    out=PP[:, jc2 * 1024:(jc2 + 1) * 1024],
                    in0=ps[:], in1=LM15D[:, blk, jc2 * 1024:(jc2 + 1) * 1024],
                    op=ALU.add)
            PPs[hx] = PP

        def emit_phase_b(hx):
            blk, h = hx // H, hx % H
            PP = PPs[hx]
            TMP = poolTM.tile([128, N], BF16, tag="TMP", name=f"TMP{hx}")
            nc.scalar.activation(TMP[:], PMD[:, blk, :], AF.Copy,
                                 scale=WBC[:, h:h + 1])
            nc.vector.tensor_tensor(out=PP[:], in0=PP[:], in1=TMP[:], op=ALU.add)
            nc.scalar.activation(PP[:], PP[:], AF.Exp, bias=NEGB[:],
                                 accum_out=D[:, hx:hx + 1])
            nc.vector.reciprocal(RD[:, hx:hx + 1], D[:, hx:hx + 1])
            nc.vector.tensor_scalar_mul(out=PP[:], in0=PP[:], scalar1=RD[:, hx:hx + 1])
            PT = poolT.tile([128, 16, 128], BF16, tag="PT", name=f"PT{hx}")
            nc.sync.dma_start(out=PT[:], in_=PP[:], transpose=True)
            PTmap[hx] = PT
            if h % 4 == 3:
                hg = h // 4
                av = psA.tile([128, 128], F32, tag="av", name="av")
                for k in range(4):
                    for jh in range(16):
                        nc.tensor.matmul(
                            av[k * 32:(k + 1) * 32, :],
                            VR[:, jh, hg * 128 + k * 32:hg * 128 + (k + 1) * 32],
                            PTmap[blk * H + hg * 4 + k][:, jh, :],
                            start=(jh == 0), stop=(jh == 15),
                            tile_position=(0, k * 32))
                if hg % 2 == 0:
                    nc.vector.tensor_copy(
                        ATT[:, hg, blk * 128:(blk + 1) * 128], av[:])
                else:
                    nc.scalar.activation(
                        ATT[:, hg, blk * 128:(blk + 1) * 128], av[:], AF.Copy)

        # -------- per-block tail: outproj, LN2, MLP -----------------------
        W1 = pool.tile([128, 2, 4 * C], BF16, tag="GSG", name="W1")
        nc.sync.dma_start(out=W1[:], in_=w1t_d[:].rearrange("(u p) c -> p u c", p=128))
        W2 = pool.tile([128, 8, C], BF16, tag="X", name="W2")
        nc.sync.dma_start(out=W2[:], in_=w2t_d[:].rearrange("(u p) c -> p u c", p=128))

        Y1T = pool.tile([128, 2, NSH], F32, tag="Y1T")
        Y1 = pool.tile([128, NBLK, C], F32, tag="Y1")
        H2 = pool.tile([128, NBLK, C], F32, tag="H2")
        H2T = pool.tile([128, 2, NSH], BF16, tag="H2T")
        M1 = pool.tile([128, 8, NSH], BF16, tag="M1")
        YT = pool.tile([128, 2, NSH], F32, tag="YT")
        OUT = pool.tile([128, NBLK, C], F32, tag="OUT")

        def emit_tail_block(blk):
            bs = slice(blk * 128, (blk + 1) * 128)
            for ch in range(2):
                op_ = psS.tile([128, 1024], F32, tag="ps", name="op")
                for u in range(2):
                    nc.tensor.matmul(
                        op_[:, :128], WO[:, u, ch * 128:(ch + 1) * 128],
                        ATT[:, u, bs],
                        start=(u == 0), stop=(u == 1))
                nc.vector.tensor_tensor(out=Y1T[:, ch, bs], in0=op_[:, :128],
                                        in1=XOT[:, ch, bs], op=ALU.add)
                tp = psT.tile([128, 128], F32, tag="tp", name="tp")
                nc.tensor.transpose(out=tp[:], in_=Y1T[:, ch, bs], identity=ident[:])
                nc.vector.tensor_copy(Y1[:, blk, ch * 128:(ch + 1) * 128], tp[:])
            r1, r2 = ln_reduce(Y1[:, blk:blk + 1, :], 1, f"c{blk}")
            rstd, nmr = ln_stats(r1, r2, 1, f"c{blk}")
            ln_normalize(H2[:, blk:blk + 1, :], Y1[:, blk:blk + 1, :], rstd, nmr, 1)
            for ch in range(2):
                tp = psT.tile([128, 128], F32, tag="tp", name="tp")
                nc.tensor.transpose(out=tp[:], in_=H2[:, blk, ch * 128:(ch + 1) * 128], identity=ident[:])
                nc.scalar.activation(H2T[:, ch, bs], tp[:], AF.Copy)
            for mc in range(8):
                mp = psS.tile([128, 1024], F32, tag="ps", name="mp")
                for u in range(2):
                    nc.tensor.matmul(
                        mp[:, :128], W1[:, u, mc * 128:(mc + 1) * 128],
                        H2T[:, u, bs],
                        start=(u == 0), stop=(u == 1))
                TM = psmall.tile([128, 128], BF16, tag="TM", name="TM")
                nc.vector.tensor_scalar_add(out=TM[:], in0=mp[:, :128], scalar1=B1[:, mc:mc + 1])
                nc.scalar.activation(M1[:, mc, bs], mp[:, :128], AF.Sigmoid, bias=B1[:, mc:mc + 1])
                nc.vector.tensor_tensor(out=M1[:, mc, bs], in0=M1[:, mc, bs], in1=TM[:], op=ALU.mult)
            for ch in range(2):
                yp = psS.tile([128, 1024], F32, tag="ps", name="yp")
                for mc in range(8):
                    nc.tensor.matmul(
                        yp[:, :128], W2[:, mc, ch * 128:(ch + 1) * 128],
                        M1[:, mc, bs],
                        start=(mc == 0), stop=(mc == 7))
                nc.vector.tensor_scalar_add(out=YT[:, ch, bs], in0=yp[:, :128], scalar1=B2[:, ch:ch + 1])
                nc.vector.tensor_tensor(out=YT[:, ch, bs], in0=YT[:, ch, bs], in1=Y1T[:, ch, bs], op=ALU.add)
                tp = psT.tile([128, 128], F32, tag="tp", name="tp")
                nc.tensor.transpose(out=tp[:], in_=YT[:, ch, bs], identity=ident[:])
                nc.vector.tensor_copy(OUT[:, blk, ch * 128:(ch + 1) * 128], tp[:])

        # phase-A/B interleaved: A(hx) then B(hx) per head, with the block-0
        # tail emitted mid-way so it overlaps block 1's softmax phase.
        for hx in range(NBLK * H):
            emit_phase_a(hx)
            emit_phase_b(hx)
            if hx == 9:
                emit_tail_block(0)
        emit_tail_block(1)

        nc.sync.dma_start(out=out_d[:].rearrange("(t p) c -> p t c", p=128), in_=OUT[:])

    nc.compile()
    return nc


# ======================= host side =======================

def _wrap16(flat):
    """dma_gather index layout: idx k at [k%16, k//16], replicated x8 groups."""
    n = flat.shape[0]
    arr = flat.reshape(n // 16, 16).T.astype(np.int16)
    return np.tile(arr, (8, 1))


def _host_prep(core, x, pair_emb, block_index, Wq, Wk, Wv, Wb, Wout, W1, b1, W2, b2):
    i0 = core * NSH
    idx = np.asarray(block_index[i0:i0 + NSH]).astype(np.int64)   # [256, 128]

    pgidx = np.zeros((128, 8, 4, 64), np.int16)
    for c in range(8):
        sub = idx[c * 32:(c + 1) * 32]                            # [32, 128]
        flat = (np.arange(32)[:, None] * 1024 + sub // 2).reshape(-1)
        for q in range(4):
            pgidx[:, c, q, :] = _wrap16(flat[q * 1024:(q + 1) * 1024])
    pgpar = np.ascontiguousarray((idx % 2).astype(np.float32).T)  # [b, i]

    lm15 = np.zeros((128, NBLK, N), ml_dtypes.bfloat16)
    scidx = np.zeros((128, NBLK, 2, 128), np.int16)
    for blk in range(NBLK):
        sub = idx[blk * 128:(blk + 1) * 128]                      # [128 i, 128 b]
        for i in range(128):
            row = sub[i]
            uniq, first_pos, counts = np.unique(row, return_index=True, return_counts=True)
            lm15[i, blk, uniq] = (np.log(counts.astype(np.float64)) + EXPB).astype(ml_dtypes.bfloat16)
            scrow = np.full(128, -1, np.int64)
            scrow[first_pos] = uniq
            for half in range(2):
                sc = np.where((scrow >= 1024 * half) & (scrow < 1024 * (half + 1)),
                              scrow - 1024 * half, -1)
                scidx[i, blk, half, :] = sc.astype(np.int16)

    scale = 1.0 / math.sqrt(DH)
    fp = np.float32
    bf = ml_dtypes.bfloat16
    feeds = {
        "x": np.ascontiguousarray(x, fp),
        "xown": np.ascontiguousarray(np.asarray(x, fp)[i0:i0 + NSH]),
        "pairs": np.ascontiguousarray(
            np.asarray(pair_emb[i0:i0 + NSH], fp).reshape(NSH * 1024, 64)),
        "pgidx": pgidx, "pgpar": pgpar, "scidx": scidx, "lm15": lm15,
        "ident": np.eye(128, dtype=fp),
        "wqt": np.ascontiguousarray(np.asarray(Wq, fp).T * scale).astype(bf),
        "wkt": np.ascontiguousarray(np.asarray(Wk, fp).T).astype(bf),
        "wvt": np.ascontiguousarray(np.asarray(Wv, fp).T).astype(bf),
        "wot": np.ascontiguousarray(np.asarray(Wout, fp).T).astype(bf),
        "w1t": np.ascontiguousarray(np.asarray(W1, fp).T).astype(bf),
        "w2t": np.ascontiguousarray(np.asarray(W2, fp).T).astype(bf),
        "b1p": np.ascontiguousarray(np.asarray(b1, fp).reshape(8, 128).T),
        "b2p": np.ascontiguousarray(np.asarray(b2, fp).reshape(2, 128).T),
        "wbc": np.tile(np.asarray(Wb, fp).reshape(1, 8) / CP, (128, 1)),
    }
    return feeds


_NC = None


def kernel(**inputs):
    global _NC
    from concourse.bass_utils import run_bass_kernel_spmd
    if _NC is None:
        _NC = build_nc()
    in_maps = [_host_prep(core, **inputs) for core in range(NCORES)]
    r = run_bass_kernel_spmd(_NC, in_maps, core_ids=list(range(NCORES)))
    out = np.concatenate([np.asarray(r.results[i]["out"]).reshape(NSH, C)
                          for i in range(NCORES)], axis=0)
    return out.astype(np.float32)


# revision 31
# speedup vs baseline: 1.2029x; 1.0542x over previous
# kernel.py — AtomTransformerBlock on 8 TRN2 NeuronCores (SPMD, no collectives).
#
# Sharding: N_atom rows across 8 cores (256 rows each); x + weights replicated
# (each core recomputes LN(x), K, V for all 2048 rows). pair_emb sharded by
# first axis. All index-derived masks are precomputed on the host (pure index
# preprocessing); all tensor math happens on device.
#
# Design — dense softmax (no score DRAM roundtrip):
#   pair-bias dma_gather (256B elements, 4 SWDGE queues in parallel; the
#   desc-gen ucode runs on Q7 core-pair `queue_num`, so 4 queues give ~4x
#   descriptor throughput) -> per-chunk channel reduce on DVE -> PMG[b, i] ->
#   xbar transpose -> local_scatter of pm values into dense-j space
#   (PMD[i, j], zeros elsewhere). Host feeds the dense log-multiplicity map
#   LM15D[i, j] = log(count)+EXPB at gathered j, 0 elsewhere. Scores stay
#   dense: S'[i,h,j] = q.k + lm15 (fused into the psum evac) + pm*wb[h], and
#   P = exp(S' - EXPB) kills non-gathered j. ACT accum_out yields the softmax
#   denominator during the exp; normalize is a per-partition tensor_scalar;
#   AV contracts dense P^T (DMA transpose) against dense V on PE with
#   col-tiled (M=32) matmuls, 4 heads per psum tile.
#   LN runs on ACT (per-chunk Copy/Square with accum_out reduces + Identity
#   scale/bias normalize) so the DVE stays free for the pair reduces, whose
#   in-order stream paces the gather phase. LN's few tiny DVE stats ops are
#   emitted between gather chunks 5 and 6 so they neither stall the gathers
#   nor wait behind the full reduce chain. The per-block tail (out-proj, LN2,
#   MLP) for block 0 is emitted mid-softmax so it overlaps block 1's phase.
import math
import os
import sys

import numpy as np

sys.path.insert(0, "/opt/trn_rl_repo")

import ml_dtypes
from contextlib import ExitStack

import concourse.bass as bass
import concourse.mybir as mybir
import concourse.tile as tile
from concourse import bacc, library_config
from concourse.tile import add_dep_helper

N, C, H, DH, CP, B = 2048, 256, 8, 32, 32, 128
NCORES = 8
NSH = N // NCORES          # 256 rows per core
NBLK = NSH // 128          # 2 i-blocks per core
F32 = mybir.dt.float32
BF16 = mybir.dt.bfloat16
I16 = mybir.dt.int16
AX = mybir.AxisListType
ALU = mybir.AluOpType
AF = mybir.ActivationFunctionType

EXPB = 12.0                # dense-softmax exclusion offset


def build_nc():
    nc = bacc.Bacc(None, target_bir_lowering=False, debug=True, num_swdge_queues=4)

    x_d = nc.declare_dram_parameter("x", [N, C], F32, isOutput=False)
    xo_d = nc.declare_dram_parameter("xown", [NSH, C], F32, isOutput=False)
    pairs_d = nc.declare_dram_parameter("pairs", [NSH * 1024, 64], F32, isOutput=False)
    pgidx_d = nc.declare_dram_parameter("pgidx", [128, 8, 4, 64], I16, isOutput=False)
    pgpar_d = nc.declare_dram_parameter("pgpar", [128, 256], F32, isOutput=False)
    scidx_d = nc.declare_dram_parameter("scidx", [128, NBLK, 2, 128], I16, isOutput=False)
    lm15_d = nc.declare_dram_parameter("lm15", [128, NBLK, N], BF16, isOutput=False)
    ident_d = nc.declare_dram_parameter("ident", [128, 128], F32, isOutput=False)
    wqt_d = nc.declare_dram_parameter("wqt", [C, C], BF16, isOutput=False)
    wkt_d = nc.declare_dram_parameter("wkt", [C, C], BF16, isOutput=False)
    wvt_d = nc.declare_dram_parameter("wvt", [C, C], BF16, isOutput=False)
    wot_d = nc.declare_dram_parameter("wot", [C, C], BF16, isOutput=False)
    w1t_d = nc.declare_dram_parameter("w1t", [C, 4 * C], BF16, isOutput=False)
    w2t_d = nc.declare_dram_parameter("w2t", [4 * C, C], BF16, isOutput=False)
    b1p_d = nc.declare_dram_parameter("b1p", [128, 8], F32, isOutput=False)
    b2p_d = nc.declare_dram_parameter("b2p", [128, 2], F32, isOutput=False)
    wbc_d = nc.declare_dram_parameter("wbc", [128, 8], F32, isOutput=False)
    out_d = nc.declare_dram_parameter("out", [NSH, C], F32, isOutput=True)

    with tile.TileContext(nc) as tc, ExitStack() as ctx:
        pool = ctx.enter_context(tc.tile_pool(name="p", bufs=1))
        psmall = ctx.enter_context(tc.tile_pool(name="psm", bufs=1))
        pool2 = ctx.enter_context(tc.tile_pool(name="p2", bufs=4))
        poolT = ctx.enter_context(tc.tile_pool(name="pt", bufs=5))
        poolTM = ctx.enter_context(tc.tile_pool(name="ptm", bufs=2))
        poolPP = ctx.enter_context(tc.tile_pool(name="pp", bufs=8))
        psT = ctx.enter_context(tc.tile_pool(name="psT", bufs=2, space="PSUM"))
        psS = ctx.enter_context(tc.tile_pool(name="psS", bufs=2, space="PSUM"))
        psA = ctx.enter_context(tc.tile_pool(name="psA", bufs=2, space="PSUM"))

        ll_mlp = nc.gpsimd.load_library(library_config.mlp)

        # ---------------- input loads (gather deps first) ----------------
        PGIDX = pool.tile([128, 8, 4, 64], I16)
        nc.sync.dma_start(out=PGIDX[:], in_=pgidx_d[:])
        ident = pool.tile([128, 128], F32)
        nc.sync.dma_start(out=ident[:], in_=ident_d[:])
        NEGB = pool.tile([128, 1], F32)
        nc.vector.memset(NEGB[:], -EXPB)
        X = pool.tile([128, 16, C], F32, tag="X")        # slot later reused by W2
        nc.sync.dma_start(out=X[:], in_=x_d[:].rearrange("(t p) c -> p t c", p=128))
        XOWN = pool.tile([128, NBLK, C], F32)
        nc.sync.dma_start(out=XOWN[:], in_=xo_d[:].rearrange("(t p) c -> p t c", p=128))
        WQ = pool.tile([128, 2, C], BF16)
        nc.sync.dma_start(out=WQ[:], in_=wqt_d[:].rearrange("(u p) c -> p u c", p=128))
        WK = pool.tile([128, 2, C], BF16)
        nc.sync.dma_start(out=WK[:], in_=wkt_d[:].rearrange("(u p) c -> p u c", p=128))
        WV = pool.tile([128, 2, C], BF16)
        nc.sync.dma_start(out=WV[:], in_=wvt_d[:].rearrange("(u p) c -> p u c", p=128))
        WO = pool.tile([128, 2, C], BF16)
        nc.sync.dma_start(out=WO[:], in_=wot_d[:].rearrange("(u p) c -> p u c", p=128))
        B1 = pool.tile([128, 8], F32)
        nc.sync.dma_start(out=B1[:], in_=b1p_d[:])
        B2 = pool.tile([128, 2], F32)
        nc.sync.dma_start(out=B2[:], in_=b2p_d[:])
        WBC = pool.tile([128, 8], F32)
        nc.sync.dma_start(out=WBC[:], in_=wbc_d[:])
        PGPAR = pool.tile([128, 256], F32)
        nc.sync.dma_start(out=PGPAR[:], in_=pgpar_d[:])
        SCIDX = pool.tile([128, NBLK, 2, 128], I16)
        nc.sync.dma_start(out=SCIDX[:], in_=scidx_d[:])
        LM15D = pool.tile([128, NBLK, N], BF16)
        nc.sync.dma_start(out=LM15D[:], in_=lm15_d[:])

        # ------- LN on ACT: accum_out reduces + Identity normalize ---------
        # DVE does only a handful of [128, nt] stats ops per LN; the caller
        # controls where those land in the DVE stream.
        def ln_reduce(src, nt, nm):
            RS1 = psmall.tile([128, nt], F32, tag=f"RS1{nm}", name=f"RS1{nm}")
            RS2 = psmall.tile([128, nt], F32, tag=f"RS2{nm}", name=f"RS2{nm}")
            for t in range(nt):
                DUM = poolTM.tile([128, C], F32, tag="DUM", name=f"DU{nm}{t}")
                nc.scalar.activation(DUM[:], src[:, t, :], AF.Copy,
                                     accum_out=RS1[:, t:t + 1])
                DUM2 = poolTM.tile([128, C], F32, tag="DUM", name=f"DV{nm}{t}")
                nc.scalar.activation(DUM2[:], src[:, t, :], AF.Square,
                                     accum_out=RS2[:, t:t + 1])
            return RS1, RS2

        def ln_stats(RS1, RS2, nt, nm):
            MU = psmall.tile([128, nt], F32, tag=f"MU{nm}", name=f"MU{nm}")
            nc.vector.tensor_scalar_mul(out=MU[:], in0=RS1[:], scalar1=1.0 / C)
            VAR = psmall.tile([128, nt], F32, tag=f"VAR{nm}", name=f"VAR{nm}")
            nc.vector.tensor_scalar_mul(out=VAR[:], in0=RS2[:], scalar1=1.0 / C)
            MSQ = psmall.tile([128, nt], F32, tag=f"MSQ{nm}", name=f"MSQ{nm}")
            nc.vector.tensor_tensor(out=MSQ[:], in0=MU[:], in1=MU[:], op=ALU.mult)
            nc.vector.tensor_tensor(out=VAR[:], in0=VAR[:], in1=MSQ[:], op=ALU.subtract)
            nc.vector.tensor_scalar_add(out=VAR[:], in0=VAR[:], scalar1=1e-5)
            RSTD = psmall.tile([128, nt], F32, tag=f"RSTD{nm}", name=f"RSTD{nm}")
            nc.vector.reciprocal(RSTD[:], VAR[:])
            nc.scalar.activation(RSTD[:], RSTD[:], AF.Sqrt)
            NMR = psmall.tile([128, nt], F32, tag=f"NMR{nm}", name=f"NMR{nm}")
            nc.vector.tensor_tensor(out=NMR[:], in0=MU[:], in1=RSTD[:], op=ALU.mult)
            nc.vector.tensor_scalar_mul(out=NMR[:], in0=NMR[:], scalar1=-1.0)
            return RSTD, NMR

        def ln_normalize(dst, src, RSTD, NMR, nt):
            for t in range(nt):
                nc.scalar.activation(dst[:, t, :], src[:, t, :], AF.Identity,
                                     scale=RSTD[:, t:t + 1], bias=NMR[:, t:t + 1])

        # ACT starts the LN reduces immediately (they only need X/XOWN).
        RS1a, RS2a = ln_reduce(X, 16, "a")
        RS1b, RS2b = ln_reduce(XOWN, NBLK, "b")

        # ----- pair gathers (4-queue) + R2 reduces on DVE ------------------
        PMG = pool.tile([128, 256], F32)                 # [b, i]; /32 folded into wbc
        PMGH = pool.tile([128, 256], BF16)
        gathers = []

        def emit_gather_chunk(cc):
            c, qh = cc // 2, cc % 2
            GPc = pool2.tile([128, 16, 64], F32, tag="GP", name=f"GP{cc}")
            for q2 in range(2):
                q = qh * 2 + q2
                g = nc.gpsimd.dma_gather(
                    out_ap=GPc[:, q2 * 8:(q2 + 1) * 8, :],
                    in_ap=pairs_d[c * 32768:(c + 1) * 32768, :],
                    idxs_ap=PGIDX[:, c, q, :],
                    num_idxs=1024,
                    num_idxs_reg=1024,
                    elem_size=64,
                    queue_num=len(gathers) % 4,
                )
                add_dep_helper(g.ins, ll_mlp.ins, reason="gather needs mlp lib")
                gathers.append(g)
            R2 = psmall.tile([128, 16, 2], F32, tag="R2", name=f"R2{cc}")
            nc.vector.reduce_sum(R2[:], GPc[:].rearrange("p i (t f) -> p i t f", t=2), axis=AX.X)
            sl = slice(cc * 16, (cc + 1) * 16)
            nc.vector.tensor_tensor(out=PMG[:, sl], in0=R2[:, :, 1], in1=R2[:, :, 0], op=ALU.subtract)
            nc.vector.tensor_tensor(out=PMG[:, sl], in0=PMG[:, sl], in1=PGPAR[:, sl], op=ALU.mult)
            nc.vector.tensor_tensor(out=PMGH[:, sl], in0=PMG[:, sl], in1=R2[:, :, 0], op=ALU.add)

        for cc in range(6):
            emit_gather_chunk(cc)

        # LN stats (tiny DVE ops) woven in here: ready once the ACT reduces
        # finish (~mid-gather), ordered after R2(5) so they never stall the
        # gather/R2 pacing at the head of the DVE stream.
        RSTDa, NMRa = ln_stats(RS1a, RS2a, 16, "a")
        RSTDb, NMRb = ln_stats(RS1b, RS2b, NBLK, "b")
        XLN = pool.tile([128, 16, C], F32, tag="GSG", name="XLN")
        ln_normalize(XLN, X, RSTDa, NMRa, 16)
        XLNO = pool.tile([128, NBLK, C], F32)
        ln_normalize(XLNO, XOWN, RSTDb, NMRb, NBLK)
        # preload the Exp ACT table before the softmax phase needs it
        EWARM = psmall.tile([128, 1], F32, tag="EWARM", name="EWARM")
        nc.scalar.activation(EWARM[:], NEGB[:], AF.Exp)

        for cc in range(6, 16):
            emit_gather_chunk(cc)

        chain_depth = int(os.environ.get("KCHAIN", "2"))
        if chain_depth > 0:
            stride = 4 * chain_depth       # per-queue ring reclaim chain
            for n in range(stride, len(gathers)):
                add_dep_helper(gathers[n].ins, gathers[n - stride].ins, sync=True,
                               reason="swdge ring reclaim chain")

        # ---------------- transposes (evacs on ACT) ----------------
        XT = pool.tile([128, 2, N], BF16, tag="XT")
        for t in range(16):
            for u in range(2):
                tp = psT.tile([128, 128], F32, tag="tp", name="tp")
                nc.tensor.transpose(out=tp[:], in_=XLN[:, t, u * 128:(u + 1) * 128], identity=ident[:])
                nc.scalar.activation(XT[:, u, t * 128:(t + 1) * 128], tp[:], AF.Copy)
        XQT = pool.tile([128, 2, NSH], BF16)
        XOT = pool.tile([128, 2, NSH], F32)
        for t in range(NBLK):
            for u in range(2):
                tp = psT.tile([128, 128], F32, tag="tp", name="tp")
                nc.tensor.transpose(out=tp[:], in_=XLNO[:, t, u * 128:(u + 1) * 128], identity=ident[:])
                nc.scalar.activation(XQT[:, u, t * 128:(t + 1) * 128], tp[:], AF.Copy)
                tp2 = psT.tile([128, 128], F32, tag="tp", name="tp2")
                nc.tensor.transpose(out=tp2[:], in_=XOWN[:, t, u * 128:(u + 1) * 128], identity=ident[:])
                nc.scalar.activation(XOT[:, u, t * 128:(t + 1) * 128], tp2[:], AF.Copy)

        # ---------------- K^T, V^T, Q^T (evacs on ACT) ----------------
        KT = pool.tile([128, 2, N], BF16, tag="KT")
        VTB = pool.tile([128, 2, N], BF16, tag="VTB")
        for ch in range(2):
            for jc in range(4):
                kp = psS.tile([128, 1024], F32, tag="ps", name="kp")
                for u in range(2):
                    nc.tensor.matmul(
                        kp[:, :512], WK[:, u, ch * 128:(ch + 1) * 128],
                        XT[:, u, jc * 512:(jc + 1) * 512],
                        start=(u == 0), stop=(u == 1))
                for u in range(2):
                    nc.tensor.matmul(
                        kp[:, 512:], WV[:, u, ch * 128:(ch + 1) * 128],
                        XT[:, u, jc * 512:(jc + 1) * 512],
                        start=(u == 0), stop=(u == 1))
                nc.scalar.activation(KT[:, ch, jc * 512:(jc + 1) * 512], kp[:, :512], AF.Copy)
                nc.scalar.activation(VTB[:, ch, jc * 512:(jc + 1) * 512], kp[:, 512:], AF.Copy)
        QT = pool.tile([128, 2, NSH], BF16)
        for ch in range(2):
            qp = psS.tile([128, 1024], F32, tag="ps", name="qp")
            for u in range(2):
                nc.tensor.matmul(
                    qp[:, :NSH], WQ[:, u, ch * 128:(ch + 1) * 128],
                    XQT[:, u, :],
                    start=(u == 0), stop=(u == 1))
            nc.scalar.activation(QT[:, ch, :], qp[:, :NSH], AF.Copy)

        VR = pool.tile([128, 16, C], BF16, tag="VR")
        for ch in range(2):
            nc.sync.dma_start(out=VR[:, :, ch * 128:(ch + 1) * 128], in_=VTB[:, ch, :], transpose=True)

        # ---------------- pm transpose (xbar) + dense scatter --------------
        pmTT = pool.tile([128, NBLK, 128], BF16)
        nc.sync.dma_start(out=pmTT[:], in_=PMGH[:], transpose=True)

        ll_ls = nc.gpsimd.load_library(library_config.local_scatter)
        for g in gathers:
            add_dep_helper(ll_ls.ins, g.ins, reason="lib switch after gathers")

        PMD = pool.tile([128, NBLK, N], BF16)
        for blk in range(NBLK):
            for half in range(2):
                ls = nc.gpsimd.local_scatter(
                    out_ap=PMD[:, blk, half * 1024:(half + 1) * 1024],
                    data_ap=pmTT[:, blk, :],
                    idxs_ap=SCIDX[:, blk, half, :],
                    channels=128,
                    num_elems=1024,
                    num_idxs=128,
                )
                add_dep_helper(ls.ins, ll_ls.ins, reason="scatter needs ls lib")

        # ---- softmax pipeline state ----
        D = pool.tile([128, NBLK * H], F32)
        RD = pool.tile([128, NBLK * H], F32)
        PPs = {}
        PTmap = {}
        ATT = pool.tile([128, 2, NSH], BF16, tag="ATT")

        def emit_phase_a(hx):
            blk, h = hx // H, hx % H
            r0 = (h % 4) * 32
            qs = QT[r0:r0 + 32, h // 4, blk * 128:(blk + 1) * 128]
            PP = poolPP.tile([128, N], BF16, tag="PP", name=f"PP{hx}")
            for jc2 in range(2):
                ps = psS.tile([128, 1024], F32, tag="ps", name=f"ps{hx}_{jc2}")
                for jc in range(2):
                    j0 = (jc2 * 2 + jc) * 512
                    nc.tensor.matmul(
                        ps[:, jc * 512:(jc + 1) * 512], qs,
                        KT[r0:r0 + 32, h // 4, j0:j0 + 512],
                        start=True, stop=True, tile_position=(r0, 0))
                nc.vector.tensor_tensor(
                    out=PP[:, jc2 * 1024:(jc2 + 1) * 1024],
                    in0=ps[:], in1=LM15D[:, blk, jc2 * 1024:(jc2 + 1) * 1024],
                    op=ALU.add)
            PPs[hx] = PP

        def emit_phase_b(hx):
            blk, h = hx // H, hx % H
            PP = PPs[hx]
            TMP = poolTM.tile([128, N], BF16, tag="TMP", name=f"TMP{hx}")
            nc.scalar.activation(TMP[:], PMD[:, blk, :], AF.Copy,
                                 scale=WBC[:, h:h + 1])
            nc.vector.tensor_tensor(out=PP[:], in0=PP[:], in1=TMP[:], op=ALU.add)
            nc.scalar.activation(PP[:], PP[:], AF.Exp, bias=NEGB[:],
                                 accum_out=D[:, hx:hx + 1])
            nc.vector.reciprocal(RD[:, hx:hx + 1], D[:, hx:hx + 1])
            nc.vector.tensor_scalar_mul(out=PP[:], in0=PP[:], scalar1=RD[:, hx:hx + 1])
            PT = poolT.tile([128, 16, 128], BF16, tag="PT", name=f"PT{hx}")
            nc.sync.dma_start(out=PT[:], in_=PP[:], transpose=True)
            PTmap[hx] = PT
            if h % 4 == 3:
                hg = h // 4
                av = psA.tile([128, 128], F32, tag="av", name="av")
                for k in range(4):
                    for jh in range(16):
                        nc.tensor.matmul(
                            av[k * 32:(k + 1) * 32, :],
                            VR[:, jh, hg * 128 + k * 32:hg * 128 + (k + 1) * 32],
                            PTmap[blk * H + hg * 4 + k][:, jh, :],
                            start=(jh == 0), stop=(jh == 15),
                            tile_position=(0, k * 32))
                if hg % 2 == 0:
                    nc.vector.tensor_copy(
                        ATT[:, hg, blk * 128:(blk + 1) * 128], av[:])
                else:
                    nc.scalar.activation(
                        ATT[:, hg, blk * 128:(blk + 1) * 128], av[:], AF.Copy)

        # -------- per-block tail: outproj, LN2, MLP -----------------------
        W1 = pool.tile([128, 2, 4 * C], BF16, tag="GSG", name="W1")
        nc.sync.dma_start(out=W1[:], in_=w1t_d[:].rearrange("(u p) c -> p u c", p=128))
        W2 = pool.tile([128, 8, C], BF16, tag="X", name="W2")
        nc.sync.dma_start(out=W2[:], in_=w2t_d[:].rearrange("(u p) c -> p u c", p=128))

        Y1T = pool.tile([128, 2, NSH], F32, tag="Y1T")
        Y1 = pool.tile([128, NBLK, C], F32, tag="Y1")
        H2 = pool.tile([128, NBLK, C], F32, tag="H2")
        H2T = pool.tile([128, 2, NSH], BF16, tag="H2T")
        M1 = pool.tile([128, 8, NSH], BF16, tag="M1")
        YT = pool.tile([128, 2, NSH], F32, tag="YT")
        OUT = pool.tile([128, NBLK, C], F32, tag="OUT")

        def emit_tail_block(blk):
            bs = slice(blk * 128, (blk + 1) * 128)
            for ch in range(2):
                op_ = psS.tile([128, 1024], F32, tag="ps", name="op")
                for u in range(2):
                    nc.tensor.matmul(
                        op_[:, :128], WO[:, u, ch * 128:(ch + 1) * 128],
                        ATT[:, u, bs],
                        start=(u == 0), stop=(u == 1))
                nc.vector.tensor_tensor(out=Y1T[:, ch, bs], in0=op_[:, :128],
                                        in1=XOT[:, ch, bs], op=ALU.add)
                tp = psT.tile([128, 128], F32, tag="tp", name="tp")
                nc.tensor.transpose(out=tp[:], in_=Y1T[:, ch, bs], identity=ident[:])
                nc.vector.tensor_copy(Y1[:, blk, ch * 128:(ch + 1) * 128], tp[:])
            r1, r2, rd1, rd2 = ln_reduce(Y1[:, blk:blk + 1, :], 1, f"c{blk}")
            rd1(); rd2()
            rstd, nmr = ln_stats(r1, r2, 1, f"c{blk}")
            ln_normalize(H2[:, blk:blk + 1, :], Y1[:, blk:blk + 1, :], rstd, nmr, 1)
            for ch in range(2):
                tp = psT.tile([128, 128], F32, tag="tp", name="tp")
                nc.tensor.transpose(out=tp[:], in_=H2[:, blk, ch * 128:(ch + 1) * 128], identity=ident[:])
                nc.scalar.activation(H2T[:, ch, bs], tp[:], AF.Copy)
            for mc in range(8):
                mp = psS.tile([128, 1024], F32, tag="ps", name="mp")
                for u in range(2):
                    nc.tensor.matmul(
                        mp[:, :128], W1[:, u, mc * 128:(mc + 1) * 128],
                        H2T[:, u, bs],
                        start=(u == 0), stop=(u == 1))
                TM = psmall.tile([128, 128], BF16, tag="TM", name="TM")
                nc.vector.tensor_scalar_add(out=TM[:], in0=mp[:, :128], scalar1=B1[:, mc:mc + 1])
                nc.scalar.activation(M1[:, mc, bs], mp[:, :128], AF.Sigmoid, bias=B1[:, mc:mc + 1])
                nc.vector.tensor_tensor(out=M1[:, mc, bs], in0=M1[:, mc, bs], in1=TM[:], op=ALU.mult)
            for ch in range(2):
                yp = psS.tile([128, 1024], F32, tag="ps", name="yp")
                for mc in range(8):
                    nc.tensor.matmul(
                        yp[:, :128], W2[:, mc, ch * 128:(ch + 1) * 128],
                        M1[:, mc, bs],
                        start=(mc == 0), stop=(mc == 7))
                nc.vector.tensor_scalar_add(out=YT[:, ch, bs], in0=yp[:, :128], scalar1=B2[:, ch:ch + 1])
                nc.vector.tensor_tensor(out=YT[:, ch, bs], in0=YT[:, ch, bs], in1=Y1T[:, ch, bs], op=ALU.add)
                tp = psT.tile([128, 128], F32, tag="tp", name="tp")
                nc.tensor.transpose(out=tp[:], in_=YT[:, ch, bs], identity=ident[:])
                nc.vector.tensor_copy(OUT[:, blk, ch * 128:(ch + 1) * 128], tp[:])

        # phase-A/B interleaved: A(hx) then B(hx) per head, with the block-0
        # tail emitted mid-way so it overlaps block 1's softmax phase.
        for hx in range(NBLK * H):
            emit_phase_a(hx)
            emit_phase_b(hx)
            if hx == 9:
                emit_tail_block(0)
        emit_tail_block(1)

        nc.sync.dma_start(out=out_d[:].rearrange("(t p) c -> p t c", p=128), in_=OUT[:])

    nc.compile()
    return nc


# ======================= host side =======================

def _wrap16(flat):
    """dma_gather index layout: idx k at [k%16, k//16], replicated x8 groups."""
    n = flat.shape[0]
    arr = flat.reshape(n // 16, 16).T.astype(np.int16)
    return np.tile(arr, (8, 1))


def _host_prep(core, x, pair_emb, block_index, Wq, Wk, Wv, Wb, Wout, W1, b1, W2, b2):
    i0 = core * NSH
    idx = np.asarray(block_index[i0:i0 + NSH]).astype(np.int64)   # [256, 128]

    pgidx = np.zeros((128, 8, 4, 64), np.int16)
    for c in range(8):
        sub = idx[c * 32:(c + 1) * 32]                            # [32, 128]
        flat = (np.arange(32)[:, None] * 1024 + sub // 2).reshape(-1)
        for q in range(4):
            pgidx[:, c, q, :] = _wrap16(flat[q * 1024:(q + 1) * 1024])
    pgpar = np.ascontiguousarray((idx % 2).astype(np.float32).T)  # [b, i]

    lm15 = np.zeros((128, NBLK, N), ml_dtypes.bfloat16)
    scidx = np.zeros((128, NBLK, 2, 128), np.int16)
    for blk in range(NBLK):
        sub = idx[blk * 128:(blk + 1) * 128]                      # [128 i, 128 b]
        for i in range(128):
            row = sub[i]
            uniq, first_pos, counts = np.unique(row, return_index=True, return_counts=True)
            lm15[i, blk, uniq] = (np.log(counts.astype(np.float64)) + EXPB).astype(ml_dtypes.bfloat16)
            scrow = np.full(128, -1, np.int64)
            scrow[first_pos] = uniq
            for half in range(2):
                sc = np.where((scrow >= 1024 * half) & (scrow < 1024 * (half + 1)),
                              scrow - 1024 * half, -1)
                scidx[i, blk, half, :] = sc.astype(np.int16)

    scale = 1.0 / math.sqrt(DH)
    fp = np.float32
    bf = ml_dtypes.bfloat16
    feeds = {
        "x": np.ascontiguousarray(x, fp),
        "xown": np.ascontiguousarray(np.asarray(x, fp)[i0:i0 + NSH]),
        "pairs": np.ascontiguousarray(
            np.asarray(pair_emb[i0:i0 + NSH], fp).reshape(NSH * 1024, 64)),
        "pgidx": pgidx, "pgpar": pgpar, "scidx": scidx, "lm15": lm15,
        "ident": np.eye(128, dtype=fp),
        "wqt": np.ascontiguousarray(np.asarray(Wq, fp).T * scale).astype(bf),
        "wkt": np.ascontiguousarray(np.asarray(Wk, fp).T).astype(bf),
        "wvt": np.ascontiguousarray(np.asarray(Wv, fp).T).astype(bf),
        "wot": np.ascontiguousarray(np.asarray(Wout, fp).T).astype(bf),
        "w1t": np.ascontiguousarray(np.asarray(W1, fp).T).astype(bf),
        "w2t": np.ascontiguousarray(np.asarray(W2, fp).T).astype(bf),
        "b1p": np.ascontiguousarray(np.asarray(b1, fp).reshape(8, 128).T),
        "b2p": np.ascontiguousarray(np.asarray(b2, fp).reshape(2, 128).T),
        "wbc": np.tile(np.asarray(Wb, fp).reshape(1, 8) / CP, (128, 1)),
    }
    return feeds


_NC = None


def kernel(**inputs):
    global _NC
    from concourse.bass_utils import run_bass_kernel_spmd
    if _NC is None:
        _NC = build_nc()
    in_maps = [_host_prep(core, **inputs) for core in range(NCORES)]
    r = run_bass_kernel_spmd(_NC, in_maps, core_ids=list(range(NCORES)))
    out = np.concatenate([np.asarray(r.results[i]["out"]).reshape(NSH, C)
                          for i in range(NCORES)], axis=0)
    return out.astype(np.float32)
